# revision 23
# baseline (speedup 1.0000x reference)
"""AWQ quantized linear (nn_AWQLinear) on 8 TRN2 NeuronCores.

  out[b,s,o] = sum_k x[b,s,k] * act_scales[k] * w[o,k] / mean(act_scales)
  w[o,k]     = (qweight[o,g,j] - zeros[o,g]) * scales[o,g],  k = 128*g + j

Strategy (column-parallel, per the AWQ tensor-parallel convention):
  - shard qweight/scales/zeros along out_features across the 8 cores
    (1376 out-features each); replicate x and act_scales.
  - per core: dequantize the weight shard on-device (ScalarE affine with
    per-partition scale/bias), transpose it to contraction-major with the
    TensorE transpose path, fold act_scales/mean into it during the
    PSUM->SBUF copy, and keep the whole [4096, 1376] bf16 wT resident in
    SBUF (11 MB).
  - stream x through TensorE: out[t,o] += xT_tile.T @ wT_tile, bf16
    operands, fp32 PSUM accumulation, 512-wide moving operand.
  - concatenate the 8 output shards on the out_features axis on the host.

Host-side work is limited to sharding/layout (slicing, transposes to
contraction-major, dtype casts) plus the scalar mean(act_scales) constant
that is folded into the replicated act_scales vector.
"""

import os

# the kernel executes on the axon/neuron jax backend; a cpu-pinned
# JAX_PLATFORMS (some harnesses set it for reference runs) would hide the
# NeuronCores from run_bass_kernel_spmd's PJRT path
if os.environ.get("JAX_PLATFORMS", "").strip() == "cpu":
    del os.environ["JAX_PLATFORMS"]

from contextlib import ExitStack

import numpy as np
import ml_dtypes

import concourse.bacc as bacc
import concourse.mybir as mybir
import concourse.tile as tile
from concourse.bass import ts
from concourse.masks import make_identity
from concourse import bass_utils

# problem shape (hardcoded per the harness contract)
B, S, IN_F, OUT_F = 4, 2048, 4096, 11008
T = B * S                      # 8192 tokens
K = IN_F                       # 4096 contraction
G = 32                         # quant groups of 128 (== partition count)
NCORES = 8
OS = OUT_F // NCORES           # 1376 out-features per core
TCH = 256                      # token chunk resident in SBUF
NTCH = T // TCH                # 32 chunks
N_OT = (OS + 127) // 128       # 11 o-tiles (last has 96 rows)
OC_CHUNKS = [(0, 512), (512, 512), (1024, 352)]  # psum-bank sized n-chunks

BF16 = mybir.dt.bfloat16
F32 = mybir.dt.float32

_CACHE = {}

# weight transpose path:
#   "mm" = regular TensorE matmul against identity (full-rate, HAM-credited)
#   "pe" = TensorE transpose-mode (uncredited ~1.2 GHz path, ~80 us slower)
W_TRANSPOSE = os.environ.get("KERNEL_WT", "mm")


def _build():
    """Emit the per-core Tile program (identical on all 8 cores)."""
    nc = bacc.Bacc("TRN2", target_bir_lowering=False, debug=False)
    xp_d = nc.dram_tensor("xp", [NTCH, 128, G, TCH], BF16, kind="ExternalInput").ap()
    qw_d = nc.dram_tensor("qw", [OS, K], mybir.dt.uint8, kind="ExternalInput").ap()
    sc_d = nc.dram_tensor("sc", [OS, G], F32, kind="ExternalInput").ap()
    zr_d = nc.dram_tensor("zr", [OS, G], F32, kind="ExternalInput").ap()
    aT_d = nc.dram_tensor("aT", [128, G], F32, kind="ExternalInput").ap()
    out_d = nc.dram_tensor("out", [T, OS], F32, kind="ExternalOutput").ap()

    with ExitStack() as ctx:
        tc = ctx.enter_context(tile.TileContext(nc))
        const = ctx.enter_context(tc.tile_pool(name="const", bufs=1))
        wres = ctx.enter_context(tc.tile_pool(name="wres", bufs=1))
        qpool = ctx.enter_context(tc.tile_pool(name="qpool", bufs=2))
        wde = ctx.enter_context(tc.tile_pool(name="wde", bufs=2))
        xpool = ctx.enter_context(tc.tile_pool(name="xpool", bufs=2))
        mpsum = ctx.enter_context(tc.tile_pool(name="mpsum", bufs=2, space="PSUM"))
        opool = ctx.enter_context(tc.tile_pool(name="opool", bufs=4))

        tpsum = ctx.enter_context(tc.tile_pool(name="tpsum", bufs=2, space="PSUM"))
        ident = const.tile([128, 128], BF16)
        make_identity(nc, ident)
        a_sb = const.tile([128, G], F32)  # act_scales/mean, contraction-major
        nc.sync.dma_start(out=a_sb, in_=aT_d)

        # resident dequantized transposed weights: [k%128, g, o] bf16
        wT = wres.tile([128, G, OS], BF16)

        # ---- weight prep: dequant (ACT) -> transpose -> a-scale ----
        for i in range(N_OT):
            P = min(128, OS - i * 128)
            q_sb = qpool.tile([128, K], mybir.dt.uint8, tag="q")
            for qi in range(4):  # column-split so dequant starts early
                nc.sync.dma_start(
                    out=q_sb[:P, ts(qi, K // 4)],
                    in_=qw_d[i * 128:i * 128 + P, ts(qi, K // 4)],
                )
            s_sb = qpool.tile([128, G], F32, tag="s")
            nc.sync.dma_start(out=s_sb[:P], in_=sc_d[i * 128:i * 128 + P, :])
            z_sb = qpool.tile([128, G], F32, tag="z")
            nc.sync.dma_start(out=z_sb[:P], in_=zr_d[i * 128:i * 128 + P, :])
            nzs = qpool.tile([128, G], F32, tag="nzs")
            nc.vector.tensor_mul(nzs[:P], z_sb[:P], s_sb[:P])
            nc.vector.tensor_scalar_mul(nzs[:P], nzs[:P], -1.0)

            wd = wde.tile([128, K], BF16)
            for g in range(G):
                # affine dequant (q*s + (-z*s)); the 479ns/op ACT table path
                # gates weight readiness if used alone, so split the groups
                # across DVE (fast tensor_scalar) and ACT to run both engines
                if g % 2 == 0:
                    nc.vector.tensor_scalar(
                        out=wd[:P, ts(g, 128)],
                        in0=q_sb[:P, ts(g, 128)],
                        scalar1=s_sb[:P, g:g + 1],
                        scalar2=nzs[:P, g:g + 1],
                        op0=mybir.AluOpType.mult,
                        op1=mybir.AluOpType.add,
                    )
                else:
                    nc.scalar.activation(
                        out=wd[:P, ts(g, 128)],
                        in_=q_sb[:P, ts(g, 128)],
                        func=mybir.ActivationFunctionType.Identity,
                        bias=nzs[:P, g:g + 1],
                        scale=s_sb[:P, g:g + 1],
                    )
            for g in range(G):
                wslice = wT[:, g, i * 128:i * 128 + P]
                if W_TRANSPOSE == "mm":
                    # wd.T via regular matmul with identity moving operand
                    pt = tpsum.tile([128, 128], F32, name="pt")
                    nc.tensor.matmul(
                        pt[:, :P], lhsT=wd[:P, ts(g, 128)], rhs=ident[:P, :P],
                        start=True, stop=True,
                    )
                else:
                    pt = tpsum.tile([128, 128], BF16, name="pt")
                    nc.tensor.transpose(pt[:, :P], wd[:P, ts(g, 128)], ident[:P, :P])
                # PSUM -> resident SBUF, folding act_scales/mean (per-partition k)
                nc.vector.tensor_scalar_mul(wslice, pt[:, :P], a_sb[:, g:g + 1])

        # ---- matmul: out[t,o] = sum_g xT[:,g,t].T @ wT[:,g,o] ----
        for tci in range(NTCH):
            xt = xpool.tile([128, G, TCH], BF16)
            nc.sync.dma_start(out=xt, in_=xp_d[tci])
            for tt in range(TCH // 128):
                ps = [
                    mpsum.tile([128, n], F32, tag=f"mm{j}", name=f"ps{j}")
                    for j, (_, n) in enumerate(OC_CHUNKS)
                ]
                for g in range(G):
                    lhsT = xt[:, g, ts(tt, 128)]
                    for j, (o0, n) in enumerate(OC_CHUNKS):
                        nc.tensor.matmul(
                            ps[j], lhsT=lhsT, rhs=wT[:, g, o0:o0 + n],
                            start=(g == 0), stop=(g == G - 1),
                        )
                ob = opool.tile([128, OS], F32)
                for j, (o0, n) in enumerate(OC_CHUNKS):
                    nc.vector.tensor_copy(ob[:, o0:o0 + n], ps[j])
                t0 = tci * TCH + tt * 128
                nc.sync.dma_start(out=out_d[t0:t0 + 128, :], in_=ob)
    nc.compile()
    return nc


def _get_program():
    if "nc" not in _CACHE:
        _CACHE["nc"] = _build()
    return _CACHE["nc"]


def kernel(x, qweight, scales, zeros, act_scales):
    x = np.asarray(x, dtype=np.float32)
    qweight = np.asarray(qweight)
    scales = np.asarray(scales, dtype=np.float32)
    zeros = np.asarray(zeros, dtype=np.float32)
    act_scales = np.asarray(act_scales, dtype=np.float32)

    # host layout prep (sharding + contraction-major repack + dtype casts)
    xp = np.ascontiguousarray(
        x.reshape(NTCH, TCH, G, 128).transpose(0, 3, 2, 1)
    ).astype(ml_dtypes.bfloat16)                       # [NTCH,128,G,TCH]
    qflat = qweight.reshape(OUT_F, K).astype(np.uint8)  # 4-bit codes, lossless
    a_vec = act_scales / act_scales.mean()
    aT = np.ascontiguousarray(a_vec.reshape(G, 128).T).astype(np.float32)

    in_maps = []
    for c in range(NCORES):
        o0 = c * OS
        in_maps.append({
            "xp": xp,
            "qw": np.ascontiguousarray(qflat[o0:o0 + OS]),
            "sc": np.ascontiguousarray(scales[o0:o0 + OS]),
            "zr": np.ascontiguousarray(zeros[o0:o0 + OS]),
            "aT": aT,
        })

    nc = _get_program()
    trace = bool(os.environ.get("KERNEL_TRACE"))
    if trace:
        try:  # register the NTFF profile hook if the image's antenv lacks it
            from antenv.axon_hooks import get_axon_ntff_profile_hook  # noqa: F401
        except ImportError:
            import sys, types, antenv  # noqa: PLC0415
            mod = types.ModuleType("antenv.axon_hooks")
            _h = [None]
            mod.set_axon_ntff_profile_hook = lambda h: _h.__setitem__(0, _h[0] or h)
            mod.get_axon_ntff_profile_hook = lambda: _h[0]
            sys.modules["antenv.axon_hooks"] = mod
            antenv.axon_hooks = mod
            from trn_agent_boot.trn_boot import _ntff_profile_via_ctypes
            mod.set_axon_ntff_profile_hook(
                _ntff_profile_via_ctypes("/opt/axon/libaxon_pjrt.so")
            )
    res = bass_utils.run_bass_kernel_spmd(
        nc, in_maps, core_ids=list(range(NCORES)), trace=trace
    )
    kernel.last_exec_time_ns = res.exec_time_ns
    if trace and res.exec_time_ns is not None:
        print(f"HW exec time: {res.exec_time_ns} ns")

    out = np.concatenate([res.results[c]["out"] for c in range(NCORES)], axis=1)
    return np.ascontiguousarray(out.reshape(B, S, OUT_F))


kernel.last_exec_time_ns = None
